# revision 1
# baseline (speedup 1.0000x reference)
"""KernelNorm2d Trainium2 Bass kernel (fp16 I/O).

Problem: x [16, 64, 256, 256] f32. 2x2 windows (stride 2) over (H, W); per-window
statistics over (C, 2, 2) = 256 elements; out = (x - mean) / sqrt(var + eps).
Data-parallel over batch: 8 cores x 2 samples each.

Host converts to fp16 (end-to-end fp16 error ~3e-4 << 2e-2 tol), halving HBM
traffic. Per-core layout: partition = window-row i (nH = 128). SBUF tile
[128(i), C=64, a=2, W=256] fp16.

Measured engine facts driving the structure:
  - DVE tensor_reduce = 1 elem/cycle, period (no fast mode for any dtype/AP
    tried). The two reduction passes (sum, sum-sq) are DVE-only ops -> DVE
    carries ~136us/core minimum. Everything else is kept OFF the DVE.
  - Normalize (per-window scalars force per-j instructions): ACT ~550ns,
    GPSIMD ~760ns per 256-elem j-column -> split between those two engines.
  - ACT Square ~0.9ns/elem makes the squares.
  - Work is quartered into (sample, w-half) units; stats/normalize/store of a
    unit overlap the reduces of later units. Loads/stores are w-half sized
    (256B runs cost some DMA efficiency but start the pipeline earlier; DMA
    is far from critical here).
"""

import os
import sys

for _p in ("/opt/trn_rl_repo", "/root/.axon_site/_ro/trn_rl_repo"):
    if os.path.isdir(_p) and _p not in sys.path:
        sys.path.append(_p)

import numpy as np

import concourse.bass as bass
import concourse.tile as tile
from concourse import bacc, mybir
from concourse.bass_utils import run_bass_kernel_spmd

# Problem constants (hardcoded per spec nn_KernelNorm2d_72164040507639)
B, C, H, W = 16, 64, 256, 256
N_CORES = 8
B_LOC = B // N_CORES          # samples per core
NH = H // 2                   # 128 window rows = partition dim
NJ = W // 2                   # 128 window cols
NJH = NJ // 2                 # window cols per w-half
WH = W // 2
EPS = 1e-5
WIN = C * 4                   # 256 elements per window
CCH = 8                       # channels per square chunk

# normalize engine split per 64-j half (v=DVE, s=ACT, g=GPSIMD)
NV, NS = 8, 18                # DVE / ACT shares; rest -> GPSIMD
# ACT Square+accum_out offload of sum-sq columns is DISABLED: a faster
# variant (NACC=32) produced NaNs, implying the cross-engine ordering of
# accum_out writes vs the DVE stats chain is not reliably enforced - the
# NACC=24 speedup was timing luck, not a tracked dependency.
NACC = 0


def _make_pattern(nv, ns, n=64):
    w = {"v": nv, "s": ns, "g": n - nv - ns}
    acc = {"v": 0.0, "s": 0.0, "g": 0.0}
    pat = []
    for k in range(n):
        best = max(w, key=lambda e: w[e] / n * (k + 1) - acc[e])
        acc[best] += 1
        pat.append(best)
    return "".join(pat)


NORM_PATTERN = _make_pattern(NV, NS)
# the final unit's normalize is the pipeline tail: spread it evenly across
# all three engines to shorten the drain
TAIL_PATTERN = _make_pattern(24, 22)


def build_kernel(debug: bool = False) -> bass.Bass:
    nc = bacc.Bacc("TRN2", debug=debug)
    f16 = mybir.dt.float16
    f32 = mybir.dt.float32
    x = nc.dram_tensor("x", [B_LOC, C, NH, 2, W], f16, kind="ExternalInput")
    y = nc.dram_tensor("y", [B_LOC, C, NH, 2, W], f16, kind="ExternalOutput")

    with tile.TileContext(nc) as tc:
        with (
            tc.tile_pool(name="data", bufs=2) as data_pool,
            tc.tile_pool(name="stats", bufs=2) as stats_pool,
            tc.tile_pool(name="scratch", bufs=2) as scratch_pool,
            tc.tile_pool(name="singles", bufs=1) as singles,
        ):
            eps_tile = singles.tile([NH, 1], f32)
            nc.vector.memset(eps_tile, EPS)

            state = {}

            def load(b, xt):
                """two c-half DMAs per sample: (a w) merges -> 1 KiB runs."""
                for ch in range(2):
                    cs = ch * (C // 2)
                    nc.sync.dma_start(
                        out=xt[:, cs : cs + C // 2],
                        in_=x[b, cs : cs + C // 2].transpose([1, 0, 2, 3]),
                    )

            def phase1(b, h, xt):
                """window sums, sums of squares, stats for one w-half."""
                ws = h * WH
                xh4 = xt[:, :, :, ws : ws + WH].rearrange(
                    "p c a (j b2) -> p j (c a) b2", b2=2
                )
                s_sum = stats_pool.tile([NH, NJH], f32, tag=f"s_sum{h}")
                nc.vector.tensor_reduce(
                    out=s_sum,
                    in_=xh4,
                    axis=mybir.AxisListType.XY,
                    op=mybir.AluOpType.add,
                )
                # sum-sq: first NJH-NACC window cols via ACT-square chunks +
                # DVE reduce; last NACC cols via per-j ACT Square+accum_out
                # (keeps those columns entirely off the DVE).
                q_sum = stats_pool.tile([NH, NJH], f32, tag=f"q_sum{h}")
                q_part = stats_pool.tile([NH, NJH], f32, tag=f"q_part{h}")
                NJC = NJH - NACC
                WC = NJC * 2
                for ci in range(C // CCH):
                    cs = ci * CCH
                    x2 = scratch_pool.tile([NH, CCH, 2, WC], f16, tag=f"x2_{h}")
                    nc.scalar.activation(
                        out=x2,
                        in_=xt[:, cs : cs + CCH, :, ws : ws + WC],
                        func=mybir.ActivationFunctionType.Square,
                    )
                    x2v = x2.rearrange("p c a (j b2) -> p j (c a) b2", b2=2)
                    tgt = q_sum[:, :NJC] if ci == 0 else q_part[:, :NJC]
                    nc.vector.tensor_reduce(
                        out=tgt,
                        in_=x2v,
                        axis=mybir.AxisListType.XY,
                        op=mybir.AluOpType.add,
                    )
                    if ci > 0:
                        nc.vector.tensor_add(
                            out=q_sum[:, :NJC],
                            in0=q_sum[:, :NJC],
                            in1=q_part[:, :NJC],
                        )
                if NACC:
                    dump = scratch_pool.tile([NH, C * 2, 2], f16, tag=f"dump{h}")
                    for j in range(NJC, NJH):
                        nc.scalar.activation(
                            out=dump,
                            in_=xh4[:, j, :, :],
                            func=mybir.ActivationFunctionType.Square,
                            accum_out=q_sum[:, j : j + 1],
                        )

                inv = stats_pool.tile([NH, NJH], f32, tag=f"inv{h}")
                tsh = stats_pool.tile([NH, NJH], f32, tag=f"tsh{h}")
                nm = stats_pool.tile([NH, NJH], f32, tag=f"nm{h}")
                var = stats_pool.tile([NH, NJH], f32, tag=f"var{h}")
                nm2 = stats_pool.tile([NH, NJH], f32, tag=f"nm2{h}")
                nc.vector.tensor_scalar_mul(out=nm, in0=s_sum, scalar1=-1.0 / WIN)
                nc.vector.tensor_mul(out=nm2, in0=nm, in1=nm)
                nc.vector.tensor_scalar_mul(out=var, in0=q_sum, scalar1=1.0 / WIN)
                nc.vector.tensor_tensor(
                    out=var, in0=var, in1=nm2, op=mybir.AluOpType.subtract
                )
                nc.scalar.activation(
                    out=var,
                    in_=var,
                    func=mybir.ActivationFunctionType.Sqrt,
                    bias=eps_tile,
                    scale=1.0,
                )
                nc.vector.reciprocal(out=inv, in_=var)
                nc.vector.tensor_mul(out=tsh, in0=nm, in1=inv)
                state[(b, h)] = (xt, inv, tsh)

            def phase2(b, h, pattern=NORM_PATTERN):
                """normalize half in place (DVE/ACT/GPSIMD split)."""
                xt, inv, tsh = state.pop((b, h))
                ws = h * WH
                xh4 = xt[:, :, :, ws : ws + WH].rearrange(
                    "p c a (j b2) -> p j (c a) b2", b2=2
                )
                for j in range(NJH):
                    win = xh4[:, j, :, :]
                    eng = pattern[j]
                    if eng == "s":
                        nc.scalar.activation(
                            out=win,
                            in_=win,
                            func=mybir.ActivationFunctionType.Identity,
                            bias=tsh[:, j : j + 1],
                            scale=inv[:, j : j + 1],
                        )
                    else:
                        e = nc.vector if eng == "v" else nc.gpsimd
                        e.tensor_scalar(
                            out=win,
                            in0=win,
                            scalar1=inv[:, j : j + 1],
                            scalar2=tsh[:, j : j + 1],
                            op0=mybir.AluOpType.mult,
                            op1=mybir.AluOpType.add,
                        )
                if h == 1:
                    nc.scalar.dma_start(
                        out=y[b].transpose([1, 0, 2, 3]), in_=xt
                    )

            # software-pipelined emission over (sample, w-half) units
            xt0 = data_pool.tile([NH, C, 2, W], f16, tag="xt")
            xt1 = data_pool.tile([NH, C, 2, W], f16, tag="xt")
            load(0, xt0)
            phase1(0, 0, xt0)
            phase1(0, 1, xt0)
            phase2(0, 0)
            load(1, xt1)
            phase1(1, 0, xt1)
            phase2(0, 1)
            phase1(1, 1, xt1)
            phase2(1, 0)
            phase2(1, 1, TAIL_PATTERN)
    nc.compile()
    return nc


_NC_CACHE = None
LAST_RESULTS = None


def _get_nc():
    global _NC_CACHE
    if _NC_CACHE is None:
        _NC_CACHE = build_kernel()
    return _NC_CACHE


def kernel(x: np.ndarray) -> np.ndarray:
    global LAST_RESULTS
    assert x.shape == (B, C, H, W), x.shape
    xh = np.ascontiguousarray(x, dtype=np.float16).reshape(B, C, NH, 2, W)
    nc = _get_nc()
    in_maps = [{"x": xh[k * B_LOC : (k + 1) * B_LOC]} for k in range(N_CORES)]
    kw = {}
    if os.environ.get("KERNEL_TRACE") == "1":
        kw["trace"] = True
        if os.environ.get("KERNEL_TRACE_DIR"):
            kw["tmpdir"] = os.environ["KERNEL_TRACE_DIR"]
    res = run_bass_kernel_spmd(nc, in_maps, core_ids=list(range(N_CORES)), **kw)
    LAST_RESULTS = res
    out = np.concatenate([r["y"] for r in res.results], axis=0)
    return out.astype(np.float32).reshape(B, C, H, W)



# revision 5
# speedup vs baseline: 1.5000x; 1.5000x over previous
"""KernelNorm2d Trainium2 Bass kernel (fp16 I/O, window-major layout).

Problem: x [16, 64, 256, 256] f32. 2x2 windows (stride 2) over (H, W); per-window
statistics over (C, 2, 2) = 256 elements; out = (x - mean) / sqrt(var + eps).
Data-parallel over batch: 8 cores x 2 samples each.

Host relayouts x to window-major [B, nH, nW, (c a b)] fp16, so each window's 256
elements are contiguous in SBUF (partition = window row i). All on-chip passes
are then contiguous-AP ops, and DMA runs are 32 KiB per partition.

Stats are ONE DVE pass via bn_stats (count/mean/M2 for even/odd halves of each
window pair), combined into mean/var with cheap batched vector math (the even/odd
groups have equal count=128, so var = mean(var_g) + var(mean_g)). This replaces
the baseline's two reduce passes + ACT squares. Normalize is per-window-column
scale+bias, split across ACT/GPSIMD (DVE joins only in the pipeline tail).
"""

import os
import sys

for _p in ("/opt/trn_rl_repo", "/root/.axon_site/_ro/trn_rl_repo"):
    if os.path.isdir(_p) and _p not in sys.path:
        sys.path.append(_p)

import numpy as np

import concourse.bass as bass
import concourse.tile as tile
from concourse import bacc, mybir
from concourse.bass_utils import run_bass_kernel_spmd

# Problem constants (hardcoded per spec nn_KernelNorm2d_72164040507639)
B, C, H, W = 16, 64, 256, 256
N_CORES = 8
B_LOC = B // N_CORES          # samples per core
NH = H // 2                   # 128 window rows = partition dim
NJ = W // 2                   # 128 window cols
WIN = C * 4                   # 256 elements per window
EPS = 1e-5
JH = NJ // 2                  # window cols per half (stats/normalize unit)

# normalize engine split per 64-j half (v=DVE, s=ACT, g=GPSIMD)
NV, NS = 0, 37                # steady state: DVE is stats-only
TV, TS = 24, 22               # tail: spread across all three engines


def _make_pattern(nv, ns, n=JH):
    w = {"v": nv, "s": ns, "g": n - nv - ns}
    acc = {"v": 0.0, "s": 0.0, "g": 0.0}
    pat = []
    for k in range(n):
        best = max(w, key=lambda e: w[e] / n * (k + 1) - acc[e])
        acc[best] += 1
        pat.append(best)
    return "".join(pat)


NORM_PATTERN = _make_pattern(NV, NS)
TAIL_PATTERN = _make_pattern(TV, TS)


def build_kernel(debug: bool = False) -> bass.Bass:
    nc = bacc.Bacc("TRN2", debug=debug)
    f16 = mybir.dt.float16
    f32 = mybir.dt.float32
    x = nc.dram_tensor("x", [B_LOC, NH, NJ, WIN], f16, kind="ExternalInput")
    y = nc.dram_tensor("y", [B_LOC, NH, NJ, WIN], f16, kind="ExternalOutput")

    with tile.TileContext(nc) as tc:
        with (
            tc.tile_pool(name="data", bufs=2) as data_pool,
            tc.tile_pool(name="stats", bufs=2) as stats_pool,
            tc.tile_pool(name="singles", bufs=1) as singles,
        ):
            eps_tile = singles.tile([NH, 1], f32)
            nc.vector.memset(eps_tile, EPS)

            state = {}

            def load(b, xt):
                """two j-half DMAs; 32 KiB contiguous per partition each."""
                for h in range(2):
                    js = h * JH
                    nc.sync.dma_start(
                        out=xt[:, js : js + JH], in_=x[b, :, js : js + JH]
                    )

            def stats(b, h, xt):
                """one bn_stats pass + combine for one j-half (JH cols)."""
                js = h * JH
                S = stats_pool.tile([NH, JH, 2, 3], f32, tag=f"S{h}")
                for t in range(JH):
                    nc.vector.bn_stats(out=S[:, t], in_=xt[:, js + t, :])
                # per-window mean/var from the two equal count=128 groups:
                # mu = Sm/2; var = Sv/256 + Sq/2 - mu^2
                m_view = S[:, :, :, 1]
                v_view = S[:, :, :, 2]
                msq = stats_pool.tile([NH, JH, 2], f32, tag=f"msq{h}")
                sm = stats_pool.tile([NH, JH], f32, tag=f"sm{h}")
                sq = stats_pool.tile([NH, JH], f32, tag=f"sq{h}")
                sv = stats_pool.tile([NH, JH], f32, tag=f"sv{h}")
                nm = stats_pool.tile([NH, JH], f32, tag=f"nm{h}")
                var = stats_pool.tile([NH, JH], f32, tag=f"var{h}")
                istd = stats_pool.tile([NH, JH], f32, tag=f"istd{h}")
                tsh = stats_pool.tile([NH, JH], f32, tag=f"tsh{h}")
                nc.scalar.activation(
                    out=msq, in_=m_view, func=mybir.ActivationFunctionType.Square
                )
                nc.vector.tensor_reduce(
                    out=sm, in_=m_view, axis=mybir.AxisListType.X,
                    op=mybir.AluOpType.add,
                )
                nc.vector.tensor_reduce(
                    out=sq, in_=msq, axis=mybir.AxisListType.X,
                    op=mybir.AluOpType.add,
                )
                nc.vector.tensor_reduce(
                    out=sv, in_=v_view, axis=mybir.AxisListType.X,
                    op=mybir.AluOpType.add,
                )
                nc.vector.tensor_scalar_mul(out=nm, in0=sm, scalar1=-0.5)
                # var = sv/256 + sq/2 - nm*nm
                nc.vector.tensor_scalar_mul(out=var, in0=sv, scalar1=1.0 / WIN)
                nc.vector.tensor_scalar_mul(out=sq, in0=sq, scalar1=0.5)
                nc.vector.tensor_add(out=var, in0=var, in1=sq)
                nc.vector.tensor_mul(out=sq, in0=nm, in1=nm)
                nc.vector.tensor_tensor(
                    out=var, in0=var, in1=sq, op=mybir.AluOpType.subtract
                )
                nc.scalar.activation(
                    out=var, in_=var, func=mybir.ActivationFunctionType.Sqrt,
                    bias=eps_tile, scale=1.0,
                )
                nc.vector.reciprocal(out=istd, in_=var)
                nc.vector.tensor_mul(out=tsh, in0=nm, in1=istd)
                state[(b, h)] = (xt, istd, tsh)

            def normalize(b, h, pattern=NORM_PATTERN):
                """normalize half in place (ACT/GPSIMD split; DVE in tail),
                then store it."""
                xt, istd, tsh = state.pop((b, h))
                js = h * JH
                for jo in range(JH):
                    win = xt[:, js + jo, :]
                    eng = pattern[jo]
                    if eng == "s":
                        nc.scalar.activation(
                            out=win,
                            in_=win,
                            func=mybir.ActivationFunctionType.Identity,
                            bias=tsh[:, jo : jo + 1],
                            scale=istd[:, jo : jo + 1],
                        )
                    else:
                        e = nc.vector if eng == "v" else nc.gpsimd
                        e.tensor_scalar(
                            out=win,
                            in0=win,
                            scalar1=istd[:, jo : jo + 1],
                            scalar2=tsh[:, jo : jo + 1],
                            op0=mybir.AluOpType.mult,
                            op1=mybir.AluOpType.add,
                        )
                nc.scalar.dma_start(
                    out=y[b, :, js : js + JH], in_=xt[:, js : js + JH]
                )

            # software-pipelined emission over (sample, j-half) units
            xt0 = data_pool.tile([NH, NJ, WIN], f16, tag="xt")
            xt1 = data_pool.tile([NH, NJ, WIN], f16, tag="xt")
            load(0, xt0)
            load(1, xt1)
            stats(0, 0, xt0)
            normalize(0, 0)
            stats(0, 1, xt0)
            normalize(0, 1)
            stats(1, 0, xt1)
            normalize(1, 0)
            stats(1, 1, xt1)
            normalize(1, 1, TAIL_PATTERN)
    nc.compile()
    return nc


_NC_CACHE = None
LAST_RESULTS = None


def _get_nc():
    global _NC_CACHE
    if _NC_CACHE is None:
        _NC_CACHE = build_kernel()
    return _NC_CACHE


def kernel(x: np.ndarray) -> np.ndarray:
    global LAST_RESULTS
    assert x.shape == (B, C, H, W), x.shape
    # window-major host relayout: [B, C, H, W] -> [B, nH, nW, (c a b)] fp16
    xh = np.ascontiguousarray(
        x.astype(np.float16)
        .reshape(B, C, NH, 2, NJ, 2)
        .transpose(0, 2, 4, 1, 3, 5)
        .reshape(B, NH, NJ, WIN)
    )
    nc = _get_nc()
    in_maps = [{"x": xh[k * B_LOC : (k + 1) * B_LOC]} for k in range(N_CORES)]
    kw = {}
    if os.environ.get("KERNEL_TRACE") == "1":
        kw["trace"] = True
        if os.environ.get("KERNEL_TRACE_DIR"):
            import tempfile

            base = os.environ["KERNEL_TRACE_DIR"]
            os.makedirs(base, exist_ok=True)
            kw["tmpdir"] = tempfile.mkdtemp(dir=base)
    res = run_bass_kernel_spmd(nc, in_maps, core_ids=list(range(N_CORES)), **kw)
    LAST_RESULTS = res
    out = np.concatenate([r["y"] for r in res.results], axis=0)
    return (
        out.reshape(B, NH, NJ, C, 2, 2)
        .transpose(0, 3, 1, 4, 2, 5)
        .reshape(B, C, H, W)
        .astype(np.float32)
    )
